# revision 27
# baseline (speedup 1.0000x reference)
"""FBank preprocessor (kaldi-style log-mel) as a Bass/Trainium2 kernel.

Pipeline per 1-sec waveform (48 kHz):
  frame (98 x 1200, hop 480) -> remove DC -> preemphasis 0.97 -> hann
  -> zero-pad 2048 -> |rfft|^2 -> mel (128 banks) -> log -> pad 98->128
  -> (x - MEAN) / (2*STD)

Everything up to the power spectrum is linear in the frame samples, so
DC-removal/preemphasis/hann/rDFT are folded into two dense (1280 x 1024)
cos/sin matrices on the host (fp8 e4m3).  The host also does the im2col:
frames are gathered, transposed to (sample-within-chunk, frame) layout
and cast to fp8, so the device runs pure compute:
  - strided DMA of per-chunk-pair moving tiles [128, 2, nf] (fp8)
  - fp8 DoubleRow matmuls against the folded DFT matrices
    (5 instrs per 128-bin tile, contract 2x128 per instr)
  - ACT Square PSUM->SBUF, mel matmul over stacked [Re^2; Im^2] (bf16)
  - clamp/log/scale epilogue, PE transpose back, DMA out

Data parallel over 8 NeuronCores: 64 waveforms each.
"""

import functools

import numpy as np
import ml_dtypes

import concourse.bass as bass
import concourse.bacc as bacc
import concourse.tile as tile
from concourse import mybir
from concourse import bass_utils

F32 = mybir.dt.float32
F32R = mybir.dt.float32r
FP8 = mybir.dt.float8e4
BF16 = mybir.dt.bfloat16

SR = 48000
WIN = 1200
HOP = 480
PADWIN = 2048
NMEL = 128
TFRAMES = 128
NFRAMES = 98
PREEMPH = 0.97
MEAN = -4.2677393
STD = 4.5689974
EPS = 1.1920928955078125e-07

NCORES = 8
B = 512
BPC = B // NCORES          # 64 waveforms per core
NCHUNK = 10                # contract chunks of 128 samples (1280 >= 1200)
NPAIR = NCHUNK // 2        # DoubleRow chunk pairs
KBINS = 1024               # rfft bins 0..1023 (bin 1024 has zero mel weight)
NW = 5                     # waveforms per block (N = 5*98 = 490 <= 512)
FTOT = BPC * NFRAMES       # 6272 frames per core

EPS_S = float(EPS * np.exp(-MEAN))
OUT_SCALE = float(1.0 / (2.0 * STD))
PADV = float((0.0 - MEAN) / (2.0 * STD))


def _mel_banks_f64():
    # torchaudio.compliance.kaldi.get_mel_banks (low 20 Hz, high nyquist)
    fft_bin_width = SR / PADWIN
    mel = lambda f: 1127.0 * np.log(1.0 + f / 700.0)
    mel_low, mel_high = mel(20.0), mel(SR / 2.0)
    delta = (mel_high - mel_low) / (NMEL + 1)
    left = mel_low + np.arange(NMEL)[:, None] * delta
    center = left + delta
    right = center + delta
    m = mel(fft_bin_width * np.arange(KBINS))[None, :]
    up = (m - left) / (center - left)
    down = (right - m) / (right - center)
    return np.maximum(0.0, np.minimum(up, down))  # (128, 1024)


def _build_host_constants():
    # T = diag(hann) @ P_preemph @ (I - ones/WIN), all (WIN x WIN), f64
    n = np.arange(WIN)
    hann = 0.5 - 0.5 * np.cos(2.0 * np.pi * n / (WIN - 1))
    T = np.eye(WIN) - np.ones((WIN, WIN)) / WIN
    P = np.eye(WIN)
    P[np.arange(1, WIN), np.arange(WIN - 1)] -= PREEMPH
    P[0, 0] -= PREEMPH            # kaldi replicate pad: first sample pairs itself
    T = P @ T
    T = hann[:, None] * T

    k = np.arange(KBINS)
    ang = 2.0 * np.pi * np.outer(n, k) / PADWIN      # (1200, 1024)
    d_re = T.T @ np.cos(ang)                          # (1200, 1024)
    d_im = T.T @ (-np.sin(ang))

    D = np.zeros((NCHUNK * 128, 2 * KBINS), np.float64)
    D[:WIN, :KBINS] = d_re
    D[:WIN, KBINS:] = d_im
    # (128 partitions = n % 128, NCHUNK, 2048 k-cols)
    dftm = D.reshape(NCHUNK, 128, 2 * KBINS).transpose(1, 0, 2)

    fbs = _mel_banks_f64() * np.exp(-MEAN)            # fold -MEAN into log arg
    fbd = np.zeros((128, 8, 128), np.float64)
    for kk in range(8):
        fbd[:, kk, :] = fbs[:, kk * 128:(kk + 1) * 128].T
    dftm = np.clip(dftm, -240.0, 240.0)
    # group-major: [128, (half, kk), chunk, 128] so each matmul group's
    # weights are one contiguous-run DMA slice
    dftg = dftm.reshape(128, NCHUNK, 2, 8, 128).transpose(0, 2, 3, 1, 4) \
        .reshape(128, 16, NCHUNK, 128)
    return (
        np.ascontiguousarray(dftg.astype(np.float32),
                             dtype=ml_dtypes.float8_e4m3),
        np.ascontiguousarray(fbd.astype(np.float32), dtype=ml_dtypes.bfloat16),
    )


def _blocks():
    # small first block so the first DFT matmuls start early, full
    # 5-waveform blocks in the middle, small last block so the final
    # serial epilogue chain (max/ln/mul/transpose/copy/DMA) is short
    out = [(0, 1)]
    b0 = 1
    while b0 < BPC:
        out.append((b0, min(NW, BPC - b0)))
        b0 += NW
    b0, nw = out[-1]
    if nw > 1:
        out[-1] = (b0, nw - 1)
        out.append((b0 + nw - 1, 1))
    return out


@functools.lru_cache(maxsize=1)
def _build_nc():
    nc = bacc.Bacc("TRN2", target_bir_lowering=False, debug=False,
                   num_devices=NCORES)

    # im2col'd moving operand: mvh[p, n1, kt, F] = frame F sample
    # (2p + kt) * 128 + n1, fp8 e4m3 (host-prepared)
    MVH = nc.dram_tensor("mvh", [NPAIR, 128, 2, FTOT], FP8,
                         kind="ExternalInput")
    DFT = nc.dram_tensor("dftm", [128, 16, NCHUNK, 128], FP8,
                         kind="ExternalInput")
    FBD = nc.dram_tensor("fbd", [128, 8, 128], BF16, kind="ExternalInput")
    IDT = nc.dram_tensor("ident", [128, 128], F32R, kind="ExternalInput")
    OUT = nc.dram_tensor("out", [BPC, TFRAMES, NMEL], F32,
                         kind="ExternalOutput")

    def out_ap(offset, dims):
        return bass.AP(tensor=OUT, offset=offset, ap=list(dims))

    with tile.TileContext(nc) as tc:
        with tc.tile_pool(name="const", bufs=1) as constp, \
             tc.tile_pool(name="mv", bufs=20) as mvp, \
             tc.tile_pool(name="sq", bufs=20) as sqp, \
             tc.tile_pool(name="epi", bufs=2) as epp, \
             tc.tile_pool(name="dft_ps", bufs=4, space="PSUM") as dftps, \
             tc.tile_pool(name="mel_ps", bufs=2, space="PSUM") as melps, \
             tc.tile_pool(name="otr_ps", bufs=1, space="PSUM") as otrps:

            ident = constp.tile([128, 128], F32R)
            nc.sync.dma_start(out=ident[:], in_=IDT.ap())

            # first block's moving tiles go out first on the sync queue; the
            # big DFT-matrix load streams per-chunk on scalar/gpsimd queues
            # so chunk-pair-0 matmuls aren't blocked behind it.
            mv_pending = {}

            def issue_mv(b0, nw):
                nf = nw * NFRAMES
                nfp = (nf + 15) // 16 * 16   # pair-plane stride % 16 == 0
                tiles = []
                for p in range(NPAIR):
                    mt = mvp.tile([128, 2, nfp], FP8, tag="mv",
                                  name=f"mv_{b0}_{p}")
                    nc.sync.dma_start(
                        out=mt[:, :, :nf],
                        in_=MVH.ap()[p][:, :,
                                        b0 * NFRAMES:b0 * NFRAMES + nf],
                    )
                    tiles.append(mt)
                mv_pending[b0] = tiles

            blocks = _blocks()
            issue_mv(*blocks[0])
            issue_mv(*blocks[1])

            # one contiguous-run DMA per matmul group, in compute order
            # (half-major), so group (h0, kk0) only gates on its own 163KB
            dftm = constp.tile([128, 16, NCHUNK, 128], FP8)
            dft_engs = [nc.scalar, nc.gpsimd]
            for g in range(16):
                dft_engs[g % 2].dma_start(out=dftm[:, g],
                                          in_=DFT.ap()[:, g])
            fbd = constp.tile([128, 8, 128], BF16)
            nc.scalar.dma_start(out=fbd[:], in_=FBD.ap())
            PADG = 8  # waveforms per pad DMA
            padt = constp.tile([TFRAMES - NFRAMES, PADG, NMEL], F32)
            nc.vector.memset(padt[:], PADV)

            for bi, (b0, nw) in enumerate(blocks):
                nf = nw * NFRAMES
                mv = mv_pending.pop(b0)
                if bi + 2 < len(blocks):
                    issue_mv(*blocks[bi + 2])

                # constant pad rows (frames 98..127): spread mid-stream on
                # the sync queue instead of bunching them in the tail
                if 3 <= bi < 3 + BPC // PADG:
                    g0 = (bi - 3) * PADG
                    nc.sync.dma_start(
                        out=out_ap(g0 * TFRAMES * NMEL + NFRAMES * NMEL,
                                   [[NMEL, TFRAMES - NFRAMES],
                                    [TFRAMES * NMEL, PADG],
                                    [1, NMEL]]),
                        in_=padt[:],
                    )

                # DFT (cos/sin folded with preprocessing) as fp8 DoubleRow
                # matmuls (2 contraction chunks per instr), power spectrum.
                # half-major order: the 8 cos groups only need the first half
                # of the DFT matrix, so block 0 starts before the sin half
                # finishes loading.
                sq_half = [[None] * 8, [None] * 8]
                pw = [None] * 8
                for half in range(2):
                    for kk in range(8):
                        g = half * 8 + kk
                        ps = dftps.tile([128, nf], F32, tag="dftps")
                        for p in range(NPAIR):
                            nc.tensor.matmul(
                                ps[:],
                                dftm[:, g, 2 * p:2 * p + 2, :],
                                mv[p][:, :, :nf],
                                start=(p == 0), stop=(p == NPAIR - 1),
                                perf_mode=mybir.MatmulPerfMode.DoubleRow,
                            )
                        st = sqp.tile([128, nf], BF16, tag="sq")
                        nc.scalar.square(st[:], ps[:])
                        sq_half[half][kk] = st
                        if half == 1:
                            pt = sqp.tile([128, nf], BF16, tag="pw")
                            nc.vector.tensor_add(pt[:], sq_half[0][kk][:],
                                                 st[:])
                            pw[kk] = pt

                # mel: contract Re^2+Im^2 (8 chunks of 128 bins)
                mel = melps.tile([128, nf], F32, tag="mel")
                for kk in range(8):
                    nc.tensor.matmul(mel[:], fbd[:, kk, :], pw[kk][:],
                                     start=(kk == 0), stop=(kk == 7))

                # log-mel + normalize: (ln(max(mel', eps')))/(2*std)
                ot = epp.tile([128, nf], F32R, tag="ot")
                nc.vector.tensor_scalar_max(ot[:], mel[:], EPS_S)
                nc.scalar.activation(ot[:], ot[:],
                                     mybir.ActivationFunctionType.Ln)
                nc.vector.tensor_scalar_mul(ot[:], ot[:], OUT_SCALE)

                # transpose back to (frames on partitions, mel on free)
                otr = otrps.tile([NFRAMES, nw * 128], F32R, tag="otr")
                for wb in range(nw):
                    nc.tensor.transpose(
                        otr[:, wb * 128:(wb + 1) * 128],
                        ot[:, wb * NFRAMES:(wb + 1) * NFRAMES],
                        ident[:],
                    )
                oc = epp.tile([NFRAMES, nw, NMEL], F32, tag="oc")
                nc.vector.tensor_copy(oc[:], otr[:].rearrange(
                    "p (w m) -> p w m", w=nw))
                nc.scalar.dma_start(
                    out=out_ap(b0 * TFRAMES * NMEL,
                               [[NMEL, NFRAMES],
                                [TFRAMES * NMEL, nw],
                                [1, NMEL]]),
                    in_=oc[:],
                )

    nc.compile()
    return nc


@functools.lru_cache(maxsize=1)
def _host_constants():
    return _build_host_constants()


def _in_maps(waveform):
    """Host-side im2col: frame, transpose to (sample-in-chunk, frame),
    cast fp8.  mvh[c][p, n1, kt, F] = wave[c*64 + F//98, 480*(F%98)
    + (2p+kt)*128 + n1]."""
    dftm, fbd = _host_constants()
    x8 = waveform.astype(ml_dtypes.float8_e4m3)
    fr = np.ascontiguousarray(np.lib.stride_tricks.as_strided(
        x8, (B, NFRAMES, NCHUNK * 128), (SR, HOP, 1)))
    mvh = np.ascontiguousarray(
        fr.reshape(NCORES, FTOT, NPAIR, 2, 128).transpose(0, 2, 4, 3, 1))
    ident = np.eye(128, dtype=np.float32)
    return [
        {"mvh": mvh[c], "dftm": dftm, "fbd": fbd, "ident": ident}
        for c in range(NCORES)
    ]


def kernel(waveform):
    waveform = np.ascontiguousarray(np.asarray(waveform, dtype=np.float32))
    assert waveform.shape == (B, SR), waveform.shape
    nc = _build_nc()
    res = bass_utils.run_bass_kernel_spmd(
        nc, _in_maps(waveform), core_ids=list(range(NCORES)), trace=False
    )
    return np.concatenate([res.results[c]["out"] for c in range(NCORES)], axis=0)


# revision 28
# speedup vs baseline: 1.0196x; 1.0196x over previous
"""FBank preprocessor (kaldi-style log-mel) as a Bass/Trainium2 kernel.

Pipeline per 1-sec waveform (48 kHz):
  frame (98 x 1200, hop 480) -> remove DC -> preemphasis 0.97 -> hann
  -> zero-pad 2048 -> |rfft|^2 -> mel (128 banks) -> log -> pad 98->128
  -> (x - MEAN) / (2*STD)

Everything up to the power spectrum is linear in the frame samples, so
DC-removal/preemphasis/hann/rDFT are folded into two dense (1280 x 1024)
cos/sin matrices on the host (fp8 e4m3).  The host also does the im2col:
frames are gathered, transposed to (sample-within-chunk, frame) layout
and cast to fp8, so the device runs pure compute:
  - strided DMA of per-chunk-pair moving tiles [128, 2, nf] (fp8)
  - fp8 DoubleRow matmuls against the folded DFT matrices
    (5 instrs per 128-bin tile, contract 2x128 per instr)
  - ACT Square PSUM->SBUF, mel matmul over stacked [Re^2; Im^2] (bf16)
  - clamp/log/scale epilogue, PE transpose back, DMA out

Data parallel over 8 NeuronCores: 64 waveforms each.
"""

import functools

import numpy as np
import ml_dtypes

import concourse.bass as bass
import concourse.bacc as bacc
import concourse.tile as tile
from concourse import mybir
from concourse import bass_utils

F32 = mybir.dt.float32
F32R = mybir.dt.float32r
FP8 = mybir.dt.float8e4
BF16 = mybir.dt.bfloat16

SR = 48000
WIN = 1200
HOP = 480
PADWIN = 2048
NMEL = 128
TFRAMES = 128
NFRAMES = 98
PREEMPH = 0.97
MEAN = -4.2677393
STD = 4.5689974
EPS = 1.1920928955078125e-07

NCORES = 8
B = 512
BPC = B // NCORES          # 64 waveforms per core
NCHUNK = 10                # contract chunks of 128 samples (1280 >= 1200)
NPAIR = NCHUNK // 2        # DoubleRow chunk pairs
KBINS = 1024               # rfft bins 0..1023 (bin 1024 has zero mel weight)
NW = 5                     # waveforms per block (N = 5*98 = 490 <= 512)
FTOT = BPC * NFRAMES       # 6272 frames per core

EPS_S = float(EPS * np.exp(-MEAN))
OUT_SCALE = float(1.0 / (2.0 * STD))
PADV = float((0.0 - MEAN) / (2.0 * STD))


def _mel_banks_f64():
    # torchaudio.compliance.kaldi.get_mel_banks (low 20 Hz, high nyquist)
    fft_bin_width = SR / PADWIN
    mel = lambda f: 1127.0 * np.log(1.0 + f / 700.0)
    mel_low, mel_high = mel(20.0), mel(SR / 2.0)
    delta = (mel_high - mel_low) / (NMEL + 1)
    left = mel_low + np.arange(NMEL)[:, None] * delta
    center = left + delta
    right = center + delta
    m = mel(fft_bin_width * np.arange(KBINS))[None, :]
    up = (m - left) / (center - left)
    down = (right - m) / (right - center)
    return np.maximum(0.0, np.minimum(up, down))  # (128, 1024)


def _build_host_constants():
    # T = diag(hann) @ P_preemph @ (I - ones/WIN), all (WIN x WIN), f64
    n = np.arange(WIN)
    hann = 0.5 - 0.5 * np.cos(2.0 * np.pi * n / (WIN - 1))
    T = np.eye(WIN) - np.ones((WIN, WIN)) / WIN
    P = np.eye(WIN)
    P[np.arange(1, WIN), np.arange(WIN - 1)] -= PREEMPH
    P[0, 0] -= PREEMPH            # kaldi replicate pad: first sample pairs itself
    T = P @ T
    T = hann[:, None] * T

    k = np.arange(KBINS)
    ang = 2.0 * np.pi * np.outer(n, k) / PADWIN      # (1200, 1024)
    d_re = T.T @ np.cos(ang)                          # (1200, 1024)
    d_im = T.T @ (-np.sin(ang))

    D = np.zeros((NCHUNK * 128, 2 * KBINS), np.float64)
    D[:WIN, :KBINS] = d_re
    D[:WIN, KBINS:] = d_im
    # (128 partitions = n % 128, NCHUNK, 2048 k-cols)
    dftm = D.reshape(NCHUNK, 128, 2 * KBINS).transpose(1, 0, 2)

    fbs = _mel_banks_f64() * np.exp(-MEAN)            # fold -MEAN into log arg
    fbd = np.zeros((128, 8, 128), np.float64)
    for kk in range(8):
        fbd[:, kk, :] = fbs[:, kk * 128:(kk + 1) * 128].T
    dftm = np.clip(dftm, -240.0, 240.0)
    # group-major: [128, (half, kk), chunk, 128] so each matmul group's
    # weights are one contiguous-run DMA slice
    dftg = dftm.reshape(128, NCHUNK, 2, 8, 128).transpose(0, 2, 3, 1, 4) \
        .reshape(128, 16, NCHUNK, 128)
    return (
        np.ascontiguousarray(dftg.astype(np.float32),
                             dtype=ml_dtypes.float8_e4m3),
        np.ascontiguousarray(fbd.astype(np.float32), dtype=ml_dtypes.bfloat16),
    )


def _blocks():
    # full 5-waveform blocks; small last block so the final serial
    # epilogue chain (max/ln/mul/transpose/copy/DMA) is short
    out = []
    b0 = 0
    while b0 < BPC:
        out.append((b0, min(NW, BPC - b0)))
        b0 += NW
    b0, nw = out[-1]
    if nw > 1:
        out[-1] = (b0, nw - 1)
        out.append((b0 + nw - 1, 1))
    return out


@functools.lru_cache(maxsize=1)
def _build_nc():
    nc = bacc.Bacc("TRN2", target_bir_lowering=False, debug=False,
                   num_devices=NCORES)

    # im2col'd moving operand: mvh[p, n1, kt, F] = frame F sample
    # (2p + kt) * 128 + n1, fp8 e4m3 (host-prepared)
    MVH = nc.dram_tensor("mvh", [NPAIR, 128, 2, FTOT], FP8,
                         kind="ExternalInput")
    DFT = nc.dram_tensor("dftm", [128, 16, NCHUNK, 128], FP8,
                         kind="ExternalInput")
    FBD = nc.dram_tensor("fbd", [128, 8, 128], BF16, kind="ExternalInput")
    IDT = nc.dram_tensor("ident", [128, 128], F32R, kind="ExternalInput")
    OUT = nc.dram_tensor("out", [BPC, TFRAMES, NMEL], F32,
                         kind="ExternalOutput")

    def out_ap(offset, dims):
        return bass.AP(tensor=OUT, offset=offset, ap=list(dims))

    with tile.TileContext(nc) as tc:
        with tc.tile_pool(name="const", bufs=1) as constp, \
             tc.tile_pool(name="mv", bufs=20) as mvp, \
             tc.tile_pool(name="sq", bufs=20) as sqp, \
             tc.tile_pool(name="epi", bufs=2) as epp, \
             tc.tile_pool(name="dft_ps", bufs=4, space="PSUM") as dftps, \
             tc.tile_pool(name="mel_ps", bufs=2, space="PSUM") as melps, \
             tc.tile_pool(name="otr_ps", bufs=1, space="PSUM") as otrps:

            ident = constp.tile([128, 128], F32R)
            nc.sync.dma_start(out=ident[:], in_=IDT.ap())

            # first block's moving tiles go out first on the sync queue; the
            # big DFT-matrix load streams per-chunk on scalar/gpsimd queues
            # so chunk-pair-0 matmuls aren't blocked behind it.
            mv_pending = {}

            def issue_mv(b0, nw):
                nf = nw * NFRAMES
                nfp = (nf + 15) // 16 * 16   # pair-plane stride % 16 == 0
                tiles = []
                for p in range(NPAIR):
                    mt = mvp.tile([128, 2, nfp], FP8, tag="mv",
                                  name=f"mv_{b0}_{p}")
                    nc.sync.dma_start(
                        out=mt[:, :, :nf],
                        in_=MVH.ap()[p][:, :,
                                        b0 * NFRAMES:b0 * NFRAMES + nf],
                    )
                    tiles.append(mt)
                mv_pending[b0] = tiles

            blocks = _blocks()
            issue_mv(*blocks[0])
            issue_mv(*blocks[1])

            # one contiguous-run DMA per matmul group, in compute order
            # (half-major), so group (h0, kk0) only gates on its own 163KB
            dftm = constp.tile([128, 16, NCHUNK, 128], FP8)
            dft_engs = [nc.scalar, nc.gpsimd]
            for g in range(16):
                dft_engs[g % 2].dma_start(out=dftm[:, g],
                                          in_=DFT.ap()[:, g])
            fbd = constp.tile([128, 8, 128], BF16)
            nc.scalar.dma_start(out=fbd[:], in_=FBD.ap())
            PADG = 8  # waveforms per pad DMA
            padt = constp.tile([TFRAMES - NFRAMES, PADG, NMEL], F32)
            nc.vector.memset(padt[:], PADV)

            for bi, (b0, nw) in enumerate(blocks):
                nf = nw * NFRAMES
                mv = mv_pending.pop(b0)
                if bi + 2 < len(blocks):
                    issue_mv(*blocks[bi + 2])

                # constant pad rows (frames 98..127): spread mid-stream on
                # the sync queue instead of bunching them in the tail
                if 3 <= bi < 3 + BPC // PADG:
                    g0 = (bi - 3) * PADG
                    nc.sync.dma_start(
                        out=out_ap(g0 * TFRAMES * NMEL + NFRAMES * NMEL,
                                   [[NMEL, TFRAMES - NFRAMES],
                                    [TFRAMES * NMEL, PADG],
                                    [1, NMEL]]),
                        in_=padt[:],
                    )

                # DFT (cos/sin folded with preprocessing) as fp8 DoubleRow
                # matmuls (2 contraction chunks per instr), power spectrum.
                # half-major order: the 8 cos groups only need the first half
                # of the DFT matrix, so block 0 starts before the sin half
                # finishes loading.
                sq_half = [[None] * 8, [None] * 8]
                pw = [None] * 8
                for half in range(2):
                    for kk in range(8):
                        g = half * 8 + kk
                        ps = dftps.tile([128, nf], F32, tag="dftps")
                        for p in range(NPAIR):
                            nc.tensor.matmul(
                                ps[:],
                                dftm[:, g, 2 * p:2 * p + 2, :],
                                mv[p][:, :, :nf],
                                start=(p == 0), stop=(p == NPAIR - 1),
                                perf_mode=mybir.MatmulPerfMode.DoubleRow,
                            )
                        st = sqp.tile([128, nf], BF16, tag="sq")
                        nc.scalar.square(st[:], ps[:])
                        sq_half[half][kk] = st
                        if half == 1:
                            pt = sqp.tile([128, nf], BF16, tag="pw")
                            nc.vector.tensor_add(pt[:], sq_half[0][kk][:],
                                                 st[:])
                            pw[kk] = pt

                # mel: contract Re^2+Im^2 (8 chunks of 128 bins)
                mel = melps.tile([128, nf], F32, tag="mel")
                for kk in range(8):
                    nc.tensor.matmul(mel[:], fbd[:, kk, :], pw[kk][:],
                                     start=(kk == 0), stop=(kk == 7))

                # log-mel + normalize: (ln(max(mel', eps')))/(2*std)
                ot = epp.tile([128, nf], F32R, tag="ot")
                nc.vector.tensor_scalar_max(ot[:], mel[:], EPS_S)
                nc.scalar.activation(ot[:], ot[:],
                                     mybir.ActivationFunctionType.Ln)
                nc.vector.tensor_scalar_mul(ot[:], ot[:], OUT_SCALE)

                # transpose back to (frames on partitions, mel on free)
                otr = otrps.tile([NFRAMES, nw * 128], F32R, tag="otr")
                for wb in range(nw):
                    nc.tensor.transpose(
                        otr[:, wb * 128:(wb + 1) * 128],
                        ot[:, wb * NFRAMES:(wb + 1) * NFRAMES],
                        ident[:],
                    )
                oc = epp.tile([NFRAMES, nw, NMEL], F32, tag="oc")
                nc.vector.tensor_copy(oc[:], otr[:].rearrange(
                    "p (w m) -> p w m", w=nw))
                nc.scalar.dma_start(
                    out=out_ap(b0 * TFRAMES * NMEL,
                               [[NMEL, NFRAMES],
                                [TFRAMES * NMEL, nw],
                                [1, NMEL]]),
                    in_=oc[:],
                )

    nc.compile()
    return nc


@functools.lru_cache(maxsize=1)
def _host_constants():
    return _build_host_constants()


def _in_maps(waveform):
    """Host-side im2col: frame, transpose to (sample-in-chunk, frame),
    cast fp8.  mvh[c][p, n1, kt, F] = wave[c*64 + F//98, 480*(F%98)
    + (2p+kt)*128 + n1]."""
    dftm, fbd = _host_constants()
    x8 = waveform.astype(ml_dtypes.float8_e4m3)
    fr = np.ascontiguousarray(np.lib.stride_tricks.as_strided(
        x8, (B, NFRAMES, NCHUNK * 128), (SR, HOP, 1)))
    mvh = np.ascontiguousarray(
        fr.reshape(NCORES, FTOT, NPAIR, 2, 128).transpose(0, 2, 4, 3, 1))
    ident = np.eye(128, dtype=np.float32)
    return [
        {"mvh": mvh[c], "dftm": dftm, "fbd": fbd, "ident": ident}
        for c in range(NCORES)
    ]


def kernel(waveform):
    waveform = np.ascontiguousarray(np.asarray(waveform, dtype=np.float32))
    assert waveform.shape == (B, SR), waveform.shape
    nc = _build_nc()
    res = bass_utils.run_bass_kernel_spmd(
        nc, _in_maps(waveform), core_ids=list(range(NCORES)), trace=False
    )
    return np.concatenate([res.results[c]["out"] for c in range(NCORES)], axis=0)


# revision 29
# speedup vs baseline: 1.0250x; 1.0053x over previous
"""FBank preprocessor (kaldi-style log-mel) as a Bass/Trainium2 kernel.

Pipeline per 1-sec waveform (48 kHz):
  frame (98 x 1200, hop 480) -> remove DC -> preemphasis 0.97 -> hann
  -> zero-pad 2048 -> |rfft|^2 -> mel (128 banks) -> log -> pad 98->128
  -> (x - MEAN) / (2*STD)

Everything up to the power spectrum is linear in the frame samples, so
DC-removal/preemphasis/hann/rDFT are folded into two dense (1280 x 1024)
cos/sin matrices on the host (fp8 e4m3).  The host also does the im2col:
frames are gathered, transposed to (sample-within-chunk, frame) layout
and cast to fp8, so the device runs pure compute:
  - strided DMA of per-chunk-pair moving tiles [128, 2, nf] (fp8)
  - fp8 DoubleRow matmuls against the folded DFT matrices
    (5 instrs per 128-bin tile, contract 2x128 per instr)
  - ACT Square PSUM->SBUF, mel matmul over stacked [Re^2; Im^2] (bf16)
  - clamp/log/scale epilogue, PE transpose back, DMA out

Data parallel over 8 NeuronCores: 64 waveforms each.
"""

import functools

import numpy as np
import ml_dtypes

import concourse.bass as bass
import concourse.bacc as bacc
import concourse.tile as tile
from concourse import mybir
from concourse import bass_utils

F32 = mybir.dt.float32
F32R = mybir.dt.float32r
FP8 = mybir.dt.float8e4
BF16 = mybir.dt.bfloat16

SR = 48000
WIN = 1200
HOP = 480
PADWIN = 2048
NMEL = 128
TFRAMES = 128
NFRAMES = 98
PREEMPH = 0.97
MEAN = -4.2677393
STD = 4.5689974
EPS = 1.1920928955078125e-07

NCORES = 8
B = 512
BPC = B // NCORES          # 64 waveforms per core
NCHUNK = 10                # contract chunks of 128 samples (1280 >= 1200)
NPAIR = NCHUNK // 2        # DoubleRow chunk pairs
KBINS = 1024               # rfft bins 0..1023 (bin 1024 has zero mel weight)
NW = 5                     # waveforms per block (N = 5*98 = 490 <= 512)
FTOT = BPC * NFRAMES       # 6272 frames per core

EPS_S = float(EPS * np.exp(-MEAN))
OUT_SCALE = float(1.0 / (2.0 * STD))
PADV = float((0.0 - MEAN) / (2.0 * STD))


def _mel_banks_f64():
    # torchaudio.compliance.kaldi.get_mel_banks (low 20 Hz, high nyquist)
    fft_bin_width = SR / PADWIN
    mel = lambda f: 1127.0 * np.log(1.0 + f / 700.0)
    mel_low, mel_high = mel(20.0), mel(SR / 2.0)
    delta = (mel_high - mel_low) / (NMEL + 1)
    left = mel_low + np.arange(NMEL)[:, None] * delta
    center = left + delta
    right = center + delta
    m = mel(fft_bin_width * np.arange(KBINS))[None, :]
    up = (m - left) / (center - left)
    down = (right - m) / (right - center)
    return np.maximum(0.0, np.minimum(up, down))  # (128, 1024)


def _build_host_constants():
    # T = diag(hann) @ P_preemph @ (I - ones/WIN), all (WIN x WIN), f64
    n = np.arange(WIN)
    hann = 0.5 - 0.5 * np.cos(2.0 * np.pi * n / (WIN - 1))
    T = np.eye(WIN) - np.ones((WIN, WIN)) / WIN
    P = np.eye(WIN)
    P[np.arange(1, WIN), np.arange(WIN - 1)] -= PREEMPH
    P[0, 0] -= PREEMPH            # kaldi replicate pad: first sample pairs itself
    T = P @ T
    T = hann[:, None] * T

    k = np.arange(KBINS)
    ang = 2.0 * np.pi * np.outer(n, k) / PADWIN      # (1200, 1024)
    d_re = T.T @ np.cos(ang)                          # (1200, 1024)
    d_im = T.T @ (-np.sin(ang))

    D = np.zeros((NCHUNK * 128, 2 * KBINS), np.float64)
    D[:WIN, :KBINS] = d_re
    D[:WIN, KBINS:] = d_im
    # (128 partitions = n % 128, NCHUNK, 2048 k-cols)
    dftm = D.reshape(NCHUNK, 128, 2 * KBINS).transpose(1, 0, 2)

    fbs = _mel_banks_f64() * np.exp(-MEAN)            # fold -MEAN into log arg
    fbd = np.zeros((128, 8, 128), np.float64)
    for kk in range(8):
        fbd[:, kk, :] = fbs[:, kk * 128:(kk + 1) * 128].T
    dftm = np.clip(dftm, -240.0, 240.0)
    # group-major: [128, (half, kk), chunk, 128] so each matmul group's
    # weights are one contiguous-run DMA slice
    dftg = dftm.reshape(128, NCHUNK, 2, 8, 128).transpose(0, 2, 3, 1, 4) \
        .reshape(128, 16, NCHUNK, 128)
    return (
        np.ascontiguousarray(dftg.astype(np.float32),
                             dtype=ml_dtypes.float8_e4m3),
        np.ascontiguousarray(fbd.astype(np.float32), dtype=ml_dtypes.bfloat16),
    )


def _blocks():
    # full 5-waveform blocks; small last block so the final serial
    # epilogue chain (max/ln/mul/transpose/copy/DMA) is short
    out = []
    b0 = 0
    while b0 < BPC:
        out.append((b0, min(NW, BPC - b0)))
        b0 += NW
    b0, nw = out[-1]
    if nw > 1:
        out[-1] = (b0, nw - 1)
        out.append((b0 + nw - 1, 1))
    return out


@functools.lru_cache(maxsize=1)
def _build_nc():
    nc = bacc.Bacc("TRN2", target_bir_lowering=False, debug=False,
                   num_devices=NCORES)

    # im2col'd moving operand: mvh[p, n1, kt, F] = frame F sample
    # (2p + kt) * 128 + n1, fp8 e4m3 (host-prepared)
    MVH = nc.dram_tensor("mvh", [NPAIR, 128, 2, FTOT], FP8,
                         kind="ExternalInput")
    DFT = nc.dram_tensor("dftm", [128, 16, NCHUNK, 128], FP8,
                         kind="ExternalInput")
    FBD = nc.dram_tensor("fbd", [128, 8, 128], BF16, kind="ExternalInput")
    IDT = nc.dram_tensor("ident", [128, 128], F32R, kind="ExternalInput")
    OUT = nc.dram_tensor("out", [BPC, TFRAMES, NMEL], F32,
                         kind="ExternalOutput")

    def out_ap(offset, dims):
        return bass.AP(tensor=OUT, offset=offset, ap=list(dims))

    with tile.TileContext(nc) as tc:
        with tc.tile_pool(name="const", bufs=1) as constp, \
             tc.tile_pool(name="mv", bufs=20) as mvp, \
             tc.tile_pool(name="sq", bufs=20) as sqp, \
             tc.tile_pool(name="epi", bufs=2) as epp, \
             tc.tile_pool(name="dft_ps", bufs=4, space="PSUM") as dftps, \
             tc.tile_pool(name="mel_ps", bufs=2, space="PSUM") as melps, \
             tc.tile_pool(name="otr_ps", bufs=1, space="PSUM") as otrps:

            ident = constp.tile([128, 128], F32R)
            nc.sync.dma_start(out=ident[:], in_=IDT.ap())

            # first block's moving tiles go out first on the sync queue; the
            # big DFT-matrix load streams per-chunk on scalar/gpsimd queues
            # so chunk-pair-0 matmuls aren't blocked behind it.
            mv_pending = {}

            def issue_mv(b0, nw, engs=None):
                nf = nw * NFRAMES
                nfp = (nf + 15) // 16 * 16   # pair-plane stride % 16 == 0
                tiles = []
                for p in range(NPAIR):
                    mt = mvp.tile([128, 2, nfp], FP8, tag="mv",
                                  name=f"mv_{b0}_{p}")
                    eng = engs[p] if engs else nc.sync
                    eng.dma_start(
                        out=mt[:, :, :nf],
                        in_=MVH.ap()[p][:, :,
                                        b0 * NFRAMES:b0 * NFRAMES + nf],
                    )
                    tiles.append(mt)
                mv_pending[b0] = tiles

            blocks = _blocks()
            # block 0's moving tiles spread over three queues, interleaved
            # with the first two DFT group slices, so the first matmul
            # group's inputs all land within ~4us
            dftm = constp.tile([128, 16, NCHUNK, 128], FP8)
            dft_engs = [nc.scalar, nc.gpsimd]
            dft_engs[0].dma_start(out=dftm[:, 0], in_=DFT.ap()[:, 0])
            dft_engs[1].dma_start(out=dftm[:, 1], in_=DFT.ap()[:, 1])
            issue_mv(*blocks[0],
                     engs=[nc.sync, nc.sync, nc.sync, nc.scalar, nc.gpsimd])
            issue_mv(*blocks[1])

            # one contiguous-run DMA per remaining matmul group, in compute
            # order (half-major)
            for g in range(2, 16):
                dft_engs[g % 2].dma_start(out=dftm[:, g],
                                          in_=DFT.ap()[:, g])
            fbd = constp.tile([128, 8, 128], BF16)
            nc.scalar.dma_start(out=fbd[:], in_=FBD.ap())
            PADG = 8  # waveforms per pad DMA
            padt = constp.tile([TFRAMES - NFRAMES, PADG, NMEL], F32)
            nc.vector.memset(padt[:], PADV)

            for bi, (b0, nw) in enumerate(blocks):
                nf = nw * NFRAMES
                mv = mv_pending.pop(b0)
                if bi + 2 < len(blocks):
                    issue_mv(*blocks[bi + 2])

                # constant pad rows (frames 98..127): spread mid-stream on
                # the sync queue instead of bunching them in the tail
                if 3 <= bi < 3 + BPC // PADG:
                    g0 = (bi - 3) * PADG
                    nc.sync.dma_start(
                        out=out_ap(g0 * TFRAMES * NMEL + NFRAMES * NMEL,
                                   [[NMEL, TFRAMES - NFRAMES],
                                    [TFRAMES * NMEL, PADG],
                                    [1, NMEL]]),
                        in_=padt[:],
                    )

                # DFT (cos/sin folded with preprocessing) as fp8 DoubleRow
                # matmuls (2 contraction chunks per instr), power spectrum.
                # half-major order: the 8 cos groups only need the first half
                # of the DFT matrix, so block 0 starts before the sin half
                # finishes loading.
                sq_half = [[None] * 8, [None] * 8]
                pw = [None] * 8
                for half in range(2):
                    for kk in range(8):
                        g = half * 8 + kk
                        ps = dftps.tile([128, nf], F32, tag="dftps")
                        for p in range(NPAIR):
                            nc.tensor.matmul(
                                ps[:],
                                dftm[:, g, 2 * p:2 * p + 2, :],
                                mv[p][:, :, :nf],
                                start=(p == 0), stop=(p == NPAIR - 1),
                                perf_mode=mybir.MatmulPerfMode.DoubleRow,
                            )
                        st = sqp.tile([128, nf], BF16, tag="sq")
                        nc.scalar.square(st[:], ps[:])
                        sq_half[half][kk] = st
                        if half == 1:
                            pt = sqp.tile([128, nf], BF16, tag="pw")
                            nc.vector.tensor_add(pt[:], sq_half[0][kk][:],
                                                 st[:])
                            pw[kk] = pt

                # mel: contract Re^2+Im^2 (8 chunks of 128 bins)
                mel = melps.tile([128, nf], F32, tag="mel")
                for kk in range(8):
                    nc.tensor.matmul(mel[:], fbd[:, kk, :], pw[kk][:],
                                     start=(kk == 0), stop=(kk == 7))

                # log-mel + normalize: (ln(max(mel', eps')))/(2*std)
                ot = epp.tile([128, nf], F32R, tag="ot")
                nc.vector.tensor_scalar_max(ot[:], mel[:], EPS_S)
                nc.scalar.activation(ot[:], ot[:],
                                     mybir.ActivationFunctionType.Ln)
                nc.vector.tensor_scalar_mul(ot[:], ot[:], OUT_SCALE)

                # transpose back to (frames on partitions, mel on free)
                otr = otrps.tile([NFRAMES, nw * 128], F32R, tag="otr")
                for wb in range(nw):
                    nc.tensor.transpose(
                        otr[:, wb * 128:(wb + 1) * 128],
                        ot[:, wb * NFRAMES:(wb + 1) * NFRAMES],
                        ident[:],
                    )
                oc = epp.tile([NFRAMES, nw, NMEL], F32, tag="oc")
                nc.vector.tensor_copy(oc[:], otr[:].rearrange(
                    "p (w m) -> p w m", w=nw))
                nc.scalar.dma_start(
                    out=out_ap(b0 * TFRAMES * NMEL,
                               [[NMEL, NFRAMES],
                                [TFRAMES * NMEL, nw],
                                [1, NMEL]]),
                    in_=oc[:],
                )

    nc.compile()
    return nc


@functools.lru_cache(maxsize=1)
def _host_constants():
    return _build_host_constants()


def _in_maps(waveform):
    """Host-side im2col: frame, transpose to (sample-in-chunk, frame),
    cast fp8.  mvh[c][p, n1, kt, F] = wave[c*64 + F//98, 480*(F%98)
    + (2p+kt)*128 + n1]."""
    dftm, fbd = _host_constants()
    x8 = waveform.astype(ml_dtypes.float8_e4m3)
    fr = np.ascontiguousarray(np.lib.stride_tricks.as_strided(
        x8, (B, NFRAMES, NCHUNK * 128), (SR, HOP, 1)))
    mvh = np.ascontiguousarray(
        fr.reshape(NCORES, FTOT, NPAIR, 2, 128).transpose(0, 2, 4, 3, 1))
    ident = np.eye(128, dtype=np.float32)
    return [
        {"mvh": mvh[c], "dftm": dftm, "fbd": fbd, "ident": ident}
        for c in range(NCORES)
    ]


def kernel(waveform):
    waveform = np.ascontiguousarray(np.asarray(waveform, dtype=np.float32))
    assert waveform.shape == (B, SR), waveform.shape
    nc = _build_nc()
    res = bass_utils.run_bass_kernel_spmd(
        nc, _in_maps(waveform), core_ids=list(range(NCORES)), trace=False
    )
    return np.concatenate([res.results[c]["out"] for c in range(NCORES)], axis=0)


# revision 30
# speedup vs baseline: 1.0283x; 1.0032x over previous
"""FBank preprocessor (kaldi-style log-mel) as a Bass/Trainium2 kernel.

Pipeline per 1-sec waveform (48 kHz):
  frame (98 x 1200, hop 480) -> remove DC -> preemphasis 0.97 -> hann
  -> zero-pad 2048 -> |rfft|^2 -> mel (128 banks) -> log -> pad 98->128
  -> (x - MEAN) / (2*STD)

Everything up to the power spectrum is linear in the frame samples, so
DC-removal/preemphasis/hann/rDFT are folded into two dense (1280 x 1024)
cos/sin matrices on the host (fp8 e4m3).  The host also does the im2col:
frames are gathered, transposed to (sample-within-chunk, frame) layout
and cast to fp8, so the device runs pure compute:
  - strided DMA of per-chunk-pair moving tiles [128, 2, nf] (fp8)
  - fp8 DoubleRow matmuls against the folded DFT matrices
    (5 instrs per 128-bin tile, contract 2x128 per instr)
  - ACT Square PSUM->SBUF, mel matmul over stacked [Re^2; Im^2] (bf16)
  - clamp/log/scale epilogue, PE transpose back, DMA out

Data parallel over 8 NeuronCores: 64 waveforms each.
"""

import functools

import numpy as np
import ml_dtypes

import concourse.bass as bass
import concourse.bacc as bacc
import concourse.tile as tile
from concourse import mybir
from concourse import bass_utils

F32 = mybir.dt.float32
F32R = mybir.dt.float32r
FP8 = mybir.dt.float8e4
BF16 = mybir.dt.bfloat16

SR = 48000
WIN = 1200
HOP = 480
PADWIN = 2048
NMEL = 128
TFRAMES = 128
NFRAMES = 98
PREEMPH = 0.97
MEAN = -4.2677393
STD = 4.5689974
EPS = 1.1920928955078125e-07

NCORES = 8
B = 512
BPC = B // NCORES          # 64 waveforms per core
NCHUNK = 10                # contract chunks of 128 samples (1280 >= 1200)
NPAIR = NCHUNK // 2        # DoubleRow chunk pairs
KBINS = 1024               # rfft bins 0..1023 (bin 1024 has zero mel weight)
NW = 5                     # waveforms per block (N = 5*98 = 490 <= 512)
FTOT = BPC * NFRAMES       # 6272 frames per core

EPS_S = float(EPS * np.exp(-MEAN))
OUT_SCALE = float(1.0 / (2.0 * STD))
PADV = float((0.0 - MEAN) / (2.0 * STD))


def _mel_banks_f64():
    # torchaudio.compliance.kaldi.get_mel_banks (low 20 Hz, high nyquist)
    fft_bin_width = SR / PADWIN
    mel = lambda f: 1127.0 * np.log(1.0 + f / 700.0)
    mel_low, mel_high = mel(20.0), mel(SR / 2.0)
    delta = (mel_high - mel_low) / (NMEL + 1)
    left = mel_low + np.arange(NMEL)[:, None] * delta
    center = left + delta
    right = center + delta
    m = mel(fft_bin_width * np.arange(KBINS))[None, :]
    up = (m - left) / (center - left)
    down = (right - m) / (right - center)
    return np.maximum(0.0, np.minimum(up, down))  # (128, 1024)


def _build_host_constants():
    # T = diag(hann) @ P_preemph @ (I - ones/WIN), all (WIN x WIN), f64
    n = np.arange(WIN)
    hann = 0.5 - 0.5 * np.cos(2.0 * np.pi * n / (WIN - 1))
    T = np.eye(WIN) - np.ones((WIN, WIN)) / WIN
    P = np.eye(WIN)
    P[np.arange(1, WIN), np.arange(WIN - 1)] -= PREEMPH
    P[0, 0] -= PREEMPH            # kaldi replicate pad: first sample pairs itself
    T = P @ T
    T = hann[:, None] * T

    k = np.arange(KBINS)
    ang = 2.0 * np.pi * np.outer(n, k) / PADWIN      # (1200, 1024)
    d_re = T.T @ np.cos(ang)                          # (1200, 1024)
    d_im = T.T @ (-np.sin(ang))

    D = np.zeros((NCHUNK * 128, 2 * KBINS), np.float64)
    D[:WIN, :KBINS] = d_re
    D[:WIN, KBINS:] = d_im
    # (128 partitions = n % 128, NCHUNK, 2048 k-cols)
    dftm = D.reshape(NCHUNK, 128, 2 * KBINS).transpose(1, 0, 2)

    fbs = _mel_banks_f64() * np.exp(-MEAN)            # fold -MEAN into log arg
    fbd = np.zeros((128, 8, 128), np.float64)
    for kk in range(8):
        fbd[:, kk, :] = fbs[:, kk * 128:(kk + 1) * 128].T
    dftm = np.clip(dftm, -240.0, 240.0)
    # group-major: [128, (half, kk), chunk, 128] so each matmul group's
    # weights are one contiguous-run DMA slice
    dftg = dftm.reshape(128, NCHUNK, 2, 8, 128).transpose(0, 2, 3, 1, 4) \
        .reshape(128, 16, NCHUNK, 128)
    return (
        np.ascontiguousarray(dftg.astype(np.float32),
                             dtype=ml_dtypes.float8_e4m3),
        np.ascontiguousarray(fbd.astype(np.float32), dtype=ml_dtypes.bfloat16),
    )


def _blocks():
    # full 5-waveform blocks; small last block so the final serial
    # epilogue chain (max/ln/mul/transpose/copy/DMA) is short
    out = []
    b0 = 0
    while b0 < BPC:
        out.append((b0, min(NW, BPC - b0)))
        b0 += NW
    b0, nw = out[-1]
    if nw > 1:
        out[-1] = (b0, nw - 1)
        out.append((b0 + nw - 1, 1))
    return out


@functools.lru_cache(maxsize=1)
def _build_nc():
    nc = bacc.Bacc("TRN2", target_bir_lowering=False, debug=False,
                   num_devices=NCORES)

    # im2col'd moving operand: mvh[p, n1, kt, F] = frame F sample
    # (2p + kt) * 128 + n1, fp8 e4m3 (host-prepared)
    MVH = nc.dram_tensor("mvh", [NPAIR, 128, 2, FTOT], FP8,
                         kind="ExternalInput")
    DFT = nc.dram_tensor("dftm", [128, 16, NCHUNK, 128], FP8,
                         kind="ExternalInput")
    FBD = nc.dram_tensor("fbd", [128, 8, 128], BF16, kind="ExternalInput")
    IDT = nc.dram_tensor("ident", [128, 128], F32R, kind="ExternalInput")
    OUT = nc.dram_tensor("out", [BPC, TFRAMES, NMEL], F32,
                         kind="ExternalOutput")

    def out_ap(offset, dims):
        return bass.AP(tensor=OUT, offset=offset, ap=list(dims))

    with tile.TileContext(nc) as tc:
        with tc.tile_pool(name="const", bufs=1) as constp, \
             tc.tile_pool(name="mv", bufs=20) as mvp, \
             tc.tile_pool(name="sq", bufs=20) as sqp, \
             tc.tile_pool(name="epi", bufs=2) as epp, \
             tc.tile_pool(name="dft_ps", bufs=4, space="PSUM") as dftps, \
             tc.tile_pool(name="mel_ps", bufs=2, space="PSUM") as melps, \
             tc.tile_pool(name="otr_ps", bufs=1, space="PSUM") as otrps:

            ident = constp.tile([128, 128], F32R)
            nc.sync.dma_start(out=ident[:], in_=IDT.ap())

            # first block's moving tiles go out first on the sync queue; the
            # big DFT-matrix load streams per-chunk on scalar/gpsimd queues
            # so chunk-pair-0 matmuls aren't blocked behind it.
            mv_pending = {}

            def issue_mv(b0, nw, engs=None):
                nf = nw * NFRAMES
                nfp = (nf + 15) // 16 * 16   # pair-plane stride % 16 == 0
                tiles = []
                for p in range(NPAIR):
                    mt = mvp.tile([128, 2, nfp], FP8, tag="mv",
                                  name=f"mv_{b0}_{p}")
                    eng = engs[p] if engs else nc.sync
                    eng.dma_start(
                        out=mt[:, :, :nf],
                        in_=MVH.ap()[p][:, :,
                                        b0 * NFRAMES:b0 * NFRAMES + nf],
                    )
                    tiles.append(mt)
                mv_pending[b0] = tiles

            blocks = _blocks()
            # block 0's moving tiles spread over three queues, interleaved
            # with the first two DFT group slices, so the first matmul
            # group's inputs all land within ~4us
            dftm = constp.tile([128, 16, NCHUNK, 128], FP8)
            dft_engs = [nc.scalar, nc.gpsimd]
            dft_engs[0].dma_start(out=dftm[:, 0], in_=DFT.ap()[:, 0])
            dft_engs[1].dma_start(out=dftm[:, 1], in_=DFT.ap()[:, 1])
            issue_mv(*blocks[0],
                     engs=[nc.sync, nc.sync, nc.sync, nc.scalar, nc.gpsimd])
            issue_mv(*blocks[1])

            # one contiguous-run DMA per remaining matmul group, in compute
            # order (half-major), over three queues (sync drains its early
            # moving tiles first but still beats the group's consume time)
            dft_engs3 = [nc.scalar, nc.gpsimd, nc.sync]
            for g in range(2, 16):
                dft_engs3[(g - 2) % 3].dma_start(out=dftm[:, g],
                                                 in_=DFT.ap()[:, g])
            fbd = constp.tile([128, 8, 128], BF16)
            nc.scalar.dma_start(out=fbd[:], in_=FBD.ap())
            PADG = 8  # waveforms per pad DMA
            padt = constp.tile([TFRAMES - NFRAMES, PADG, NMEL], F32)
            nc.vector.memset(padt[:], PADV)

            for bi, (b0, nw) in enumerate(blocks):
                nf = nw * NFRAMES
                mv = mv_pending.pop(b0)
                if bi + 2 < len(blocks):
                    issue_mv(*blocks[bi + 2])

                # constant pad rows (frames 98..127): spread mid-stream on
                # the sync queue instead of bunching them in the tail
                if 3 <= bi < 3 + BPC // PADG:
                    g0 = (bi - 3) * PADG
                    nc.sync.dma_start(
                        out=out_ap(g0 * TFRAMES * NMEL + NFRAMES * NMEL,
                                   [[NMEL, TFRAMES - NFRAMES],
                                    [TFRAMES * NMEL, PADG],
                                    [1, NMEL]]),
                        in_=padt[:],
                    )

                # DFT (cos/sin folded with preprocessing) as fp8 DoubleRow
                # matmuls (2 contraction chunks per instr), power spectrum.
                # half-major order: the 8 cos groups only need the first half
                # of the DFT matrix, so block 0 starts before the sin half
                # finishes loading.
                sq_half = [[None] * 8, [None] * 8]
                pw = [None] * 8
                for half in range(2):
                    for kk in range(8):
                        g = half * 8 + kk
                        ps = dftps.tile([128, nf], F32, tag="dftps")
                        for p in range(NPAIR):
                            nc.tensor.matmul(
                                ps[:],
                                dftm[:, g, 2 * p:2 * p + 2, :],
                                mv[p][:, :, :nf],
                                start=(p == 0), stop=(p == NPAIR - 1),
                                perf_mode=mybir.MatmulPerfMode.DoubleRow,
                            )
                        st = sqp.tile([128, nf], BF16, tag="sq")
                        nc.scalar.square(st[:], ps[:])
                        sq_half[half][kk] = st
                        if half == 1:
                            pt = sqp.tile([128, nf], BF16, tag="pw")
                            nc.vector.tensor_add(pt[:], sq_half[0][kk][:],
                                                 st[:])
                            pw[kk] = pt

                # mel: contract Re^2+Im^2 (8 chunks of 128 bins)
                mel = melps.tile([128, nf], F32, tag="mel")
                for kk in range(8):
                    nc.tensor.matmul(mel[:], fbd[:, kk, :], pw[kk][:],
                                     start=(kk == 0), stop=(kk == 7))

                # log-mel + normalize: (ln(max(mel', eps')))/(2*std)
                ot = epp.tile([128, nf], F32R, tag="ot")
                nc.vector.tensor_scalar_max(ot[:], mel[:], EPS_S)
                nc.scalar.activation(ot[:], ot[:],
                                     mybir.ActivationFunctionType.Ln)
                nc.vector.tensor_scalar_mul(ot[:], ot[:], OUT_SCALE)

                # transpose back to (frames on partitions, mel on free)
                otr = otrps.tile([NFRAMES, nw * 128], F32R, tag="otr")
                for wb in range(nw):
                    nc.tensor.transpose(
                        otr[:, wb * 128:(wb + 1) * 128],
                        ot[:, wb * NFRAMES:(wb + 1) * NFRAMES],
                        ident[:],
                    )
                oc = epp.tile([NFRAMES, nw, NMEL], F32, tag="oc")
                nc.vector.tensor_copy(oc[:], otr[:].rearrange(
                    "p (w m) -> p w m", w=nw))
                nc.scalar.dma_start(
                    out=out_ap(b0 * TFRAMES * NMEL,
                               [[NMEL, NFRAMES],
                                [TFRAMES * NMEL, nw],
                                [1, NMEL]]),
                    in_=oc[:],
                )

    nc.compile()
    return nc


@functools.lru_cache(maxsize=1)
def _host_constants():
    return _build_host_constants()


def _in_maps(waveform):
    """Host-side im2col: frame, transpose to (sample-in-chunk, frame),
    cast fp8.  mvh[c][p, n1, kt, F] = wave[c*64 + F//98, 480*(F%98)
    + (2p+kt)*128 + n1]."""
    dftm, fbd = _host_constants()
    x8 = waveform.astype(ml_dtypes.float8_e4m3)
    fr = np.ascontiguousarray(np.lib.stride_tricks.as_strided(
        x8, (B, NFRAMES, NCHUNK * 128), (SR, HOP, 1)))
    mvh = np.ascontiguousarray(
        fr.reshape(NCORES, FTOT, NPAIR, 2, 128).transpose(0, 2, 4, 3, 1))
    ident = np.eye(128, dtype=np.float32)
    return [
        {"mvh": mvh[c], "dftm": dftm, "fbd": fbd, "ident": ident}
        for c in range(NCORES)
    ]


def kernel(waveform):
    waveform = np.ascontiguousarray(np.asarray(waveform, dtype=np.float32))
    assert waveform.shape == (B, SR), waveform.shape
    nc = _build_nc()
    res = bass_utils.run_bass_kernel_spmd(
        nc, _in_maps(waveform), core_ids=list(range(NCORES)), trace=False
    )
    return np.concatenate([res.results[c]["out"] for c in range(NCORES)], axis=0)


# revision 34
# speedup vs baseline: 1.0622x; 1.0330x over previous
"""FBank preprocessor (kaldi-style log-mel) as a Bass/Trainium2 kernel.

Pipeline per 1-sec waveform (48 kHz):
  frame (98 x 1200, hop 480) -> remove DC -> preemphasis 0.97 -> hann
  -> zero-pad 2048 -> |rfft|^2 -> mel (128 banks) -> log -> pad 98->128
  -> (x - MEAN) / (2*STD)

Everything up to the power spectrum is linear in the frame samples, so
DC-removal/preemphasis/hann/rDFT are folded into two dense (1280 x 1024)
cos/sin matrices on the host (fp8 e4m3).  The host also does the im2col:
frames are gathered, transposed to (sample-within-chunk, frame) layout
and cast to fp8, so the device runs pure compute:
  - strided DMA of per-chunk-pair moving tiles [128, 2, nf] (fp8)
  - fp8 DoubleRow matmuls against the folded DFT matrices
    (5 instrs per 128-bin tile, contract 2x128 per instr)
  - ACT Square PSUM->SBUF, mel matmul over stacked [Re^2; Im^2] (bf16)
  - clamp/log/scale epilogue, PE transpose back, DMA out

Data parallel over 8 NeuronCores: 64 waveforms each.
"""

import functools

import numpy as np
import ml_dtypes

import concourse.bass as bass
import concourse.bacc as bacc
import concourse.tile as tile
from concourse import mybir
from concourse import bass_utils

F32 = mybir.dt.float32
F32R = mybir.dt.float32r
FP8 = mybir.dt.float8e4
BF16 = mybir.dt.bfloat16

SR = 48000
WIN = 1200
HOP = 480
PADWIN = 2048
NMEL = 128
TFRAMES = 128
NFRAMES = 98
PREEMPH = 0.97
MEAN = -4.2677393
STD = 4.5689974
EPS = 1.1920928955078125e-07

NCORES = 8
B = 512
BPC = B // NCORES          # 64 waveforms per core
NCHUNK = 10                # contract chunks of 128 samples (1280 >= 1200)
NPAIR = NCHUNK // 2        # DoubleRow chunk pairs
KBINS = 1024               # rfft bins 0..1023 (bin 1024 has zero mel weight)
NW = 5                     # waveforms per block (N = 5*98 = 490 <= 512)
FTOT = BPC * NFRAMES       # 6272 frames per core

EPS_S = float(EPS * np.exp(-MEAN))
OUT_SCALE = float(1.0 / (2.0 * STD))
PADV = float((0.0 - MEAN) / (2.0 * STD))


def _mel_banks_f64():
    # torchaudio.compliance.kaldi.get_mel_banks (low 20 Hz, high nyquist)
    fft_bin_width = SR / PADWIN
    mel = lambda f: 1127.0 * np.log(1.0 + f / 700.0)
    mel_low, mel_high = mel(20.0), mel(SR / 2.0)
    delta = (mel_high - mel_low) / (NMEL + 1)
    left = mel_low + np.arange(NMEL)[:, None] * delta
    center = left + delta
    right = center + delta
    m = mel(fft_bin_width * np.arange(KBINS))[None, :]
    up = (m - left) / (center - left)
    down = (right - m) / (right - center)
    return np.maximum(0.0, np.minimum(up, down))  # (128, 1024)


def _build_host_constants():
    # T = diag(hann) @ P_preemph @ (I - ones/WIN), all (WIN x WIN), f64
    n = np.arange(WIN)
    hann = 0.5 - 0.5 * np.cos(2.0 * np.pi * n / (WIN - 1))
    T = np.eye(WIN) - np.ones((WIN, WIN)) / WIN
    P = np.eye(WIN)
    P[np.arange(1, WIN), np.arange(WIN - 1)] -= PREEMPH
    P[0, 0] -= PREEMPH            # kaldi replicate pad: first sample pairs itself
    T = P @ T
    T = hann[:, None] * T

    k = np.arange(KBINS)
    ang = 2.0 * np.pi * np.outer(n, k) / PADWIN      # (1200, 1024)
    d_re = T.T @ np.cos(ang)                          # (1200, 1024)
    d_im = T.T @ (-np.sin(ang))

    D = np.zeros((NCHUNK * 128, 2 * KBINS), np.float64)
    D[:WIN, :KBINS] = d_re
    D[:WIN, KBINS:] = d_im
    # (128 partitions = n % 128, NCHUNK, 2048 k-cols)
    dftm = D.reshape(NCHUNK, 128, 2 * KBINS).transpose(1, 0, 2)

    fbs = _mel_banks_f64() * np.exp(-MEAN)            # fold -MEAN into log arg
    fbd = np.zeros((128, 8, 128), np.float64)
    for kk in range(8):
        fbd[:, kk, :] = fbs[:, kk * 128:(kk + 1) * 128].T
    dftm = np.clip(dftm, -240.0, 240.0)
    # group-major: [128, (half, kk), chunk, 128] so each matmul group's
    # weights are one contiguous-run DMA slice
    dftg = dftm.reshape(128, NCHUNK, 2, 8, 128).transpose(0, 2, 3, 1, 4) \
        .reshape(128, 16, NCHUNK, 128)
    return (
        np.ascontiguousarray(dftg.astype(np.float32),
                             dtype=ml_dtypes.float8_e4m3),
        np.ascontiguousarray(fbd.astype(np.float32), dtype=ml_dtypes.bfloat16),
    )


def _blocks():
    # full 5-waveform blocks; small last block so the final serial
    # epilogue chain (max/ln/mul/transpose/copy/DMA) is short
    out = []
    b0 = 0
    while b0 < BPC:
        out.append((b0, min(NW, BPC - b0)))
        b0 += NW
    b0, nw = out[-1]
    if nw > 1:
        out[-1] = (b0, nw - 1)
        out.append((b0 + nw - 1, 1))
    return out


@functools.lru_cache(maxsize=1)
def _build_nc():
    nc = bacc.Bacc("TRN2", target_bir_lowering=False, debug=False,
                   num_devices=NCORES)

    # im2col'd moving operand: mvh[p, n1, kt, F] = frame F sample
    # (2p + kt) * 128 + n1, fp8 e4m3 (host-prepared)
    MVH = nc.dram_tensor("mvh", [NPAIR, 128, 2, FTOT], FP8,
                         kind="ExternalInput")
    DFT = nc.dram_tensor("dftm", [128, 16, NCHUNK, 128], FP8,
                         kind="ExternalInput")
    FBD = nc.dram_tensor("fbd", [128, 8, 128], BF16, kind="ExternalInput")
    # output in device-natural layout: mel on partitions, frames on free;
    # the host transposes to (wv, frame, mel) and adds the constant pad rows
    OUT = nc.dram_tensor("out", [128, FTOT], F32, kind="ExternalOutput")

    with tile.TileContext(nc) as tc:
        with tc.tile_pool(name="const", bufs=1) as constp, \
             tc.tile_pool(name="mv", bufs=20) as mvp, \
             tc.tile_pool(name="sq", bufs=20) as sqp, \
             tc.tile_pool(name="epi", bufs=3) as epp, \
             tc.tile_pool(name="dft_ps", bufs=6, space="PSUM") as dftps, \
             tc.tile_pool(name="mel_ps", bufs=2, space="PSUM") as melps:

            # first block's moving tiles go out first on the sync queue; the
            # big DFT-matrix load streams per-chunk on scalar/gpsimd queues
            # so chunk-pair-0 matmuls aren't blocked behind it.
            mv_pending = {}

            def issue_mv(b0, nw, engs=None):
                nf = nw * NFRAMES
                nfp = (nf + 15) // 16 * 16   # pair-plane stride % 16 == 0
                tiles = []
                for p in range(NPAIR):
                    mt = mvp.tile([128, 2, nfp], FP8, tag="mv",
                                  name=f"mv_{b0}_{p}")
                    eng = engs[p] if engs else nc.sync
                    eng.dma_start(
                        out=mt[:, :, :nf],
                        in_=MVH.ap()[p][:, :,
                                        b0 * NFRAMES:b0 * NFRAMES + nf],
                    )
                    tiles.append(mt)
                mv_pending[b0] = tiles

            blocks = _blocks()
            # block 0's moving tiles spread over three queues, interleaved
            # with the first two DFT group slices, so the first matmul
            # group's inputs all land within ~4us
            dftm = constp.tile([128, 16, NCHUNK, 128], FP8)
            dft_engs = [nc.scalar, nc.gpsimd]
            dft_engs[0].dma_start(out=dftm[:, 0], in_=DFT.ap()[:, 0])
            dft_engs[1].dma_start(out=dftm[:, 1], in_=DFT.ap()[:, 1])
            issue_mv(*blocks[0],
                     engs=[nc.sync, nc.sync, nc.sync, nc.scalar, nc.gpsimd])
            issue_mv(*blocks[1])

            # one contiguous-run DMA per remaining matmul group, in compute
            # order (half-major), over three queues (sync drains its early
            # moving tiles first but still beats the group's consume time)
            dft_engs3 = [nc.scalar, nc.gpsimd, nc.sync]
            for g in range(2, 16):
                dft_engs3[(g - 2) % 3].dma_start(out=dftm[:, g],
                                                 in_=DFT.ap()[:, g])
            fbd = constp.tile([128, 8, 128], BF16)
            nc.scalar.dma_start(out=fbd[:], in_=FBD.ap())

            for bi, (b0, nw) in enumerate(blocks):
                nf = nw * NFRAMES
                mv = mv_pending.pop(b0)
                if bi + 2 < len(blocks):
                    issue_mv(*blocks[bi + 2])

                # DFT (cos/sin folded with preprocessing) as fp8 DoubleRow
                # matmuls (2 contraction chunks per instr), power spectrum.
                # half-major order: the 8 cos groups only need the first half
                # of the DFT matrix, so block 0 starts before the sin half
                # finishes loading.
                sq_half = [[None] * 8, [None] * 8]
                pw = [None] * 8
                for half in range(2):
                    for kk in range(8):
                        g = half * 8 + kk
                        ps = dftps.tile([128, nf], F32, tag="dftps")
                        for p in range(NPAIR):
                            nc.tensor.matmul(
                                ps[:],
                                dftm[:, g, 2 * p:2 * p + 2, :],
                                mv[p][:, :, :nf],
                                start=(p == 0), stop=(p == NPAIR - 1),
                                perf_mode=mybir.MatmulPerfMode.DoubleRow,
                            )
                        st = sqp.tile([128, nf], BF16, tag="sq")
                        nc.scalar.square(st[:], ps[:])
                        sq_half[half][kk] = st
                        if half == 1:
                            pt = sqp.tile([128, nf], BF16, tag="pw")
                            nc.vector.tensor_add(pt[:], sq_half[0][kk][:],
                                                 st[:])
                            pw[kk] = pt

                # mel: contract Re^2+Im^2 (8 chunks of 128 bins)
                mel = melps.tile([128, nf], F32, tag="mel")
                for kk in range(8):
                    nc.tensor.matmul(mel[:], fbd[:, kk, :], pw[kk][:],
                                     start=(kk == 0), stop=(kk == 7))

                # log-mel + normalize: (ln(max(mel', eps')))/(2*std),
                # written out in [mel, frame] layout (host transposes)
                ot = epp.tile([128, nf], F32, tag="ot")
                nc.vector.tensor_scalar_max(ot[:], mel[:], EPS_S)
                nc.scalar.activation(ot[:], ot[:],
                                     mybir.ActivationFunctionType.Ln)
                nc.vector.tensor_scalar_mul(ot[:], ot[:], OUT_SCALE)
                nc.scalar.dma_start(
                    out=OUT.ap()[:, b0 * NFRAMES:b0 * NFRAMES + nf],
                    in_=ot[:],
                )

    nc.compile()
    return nc


@functools.lru_cache(maxsize=1)
def _host_constants():
    return _build_host_constants()


def _in_maps(waveform):
    """Host-side im2col: frame, transpose to (sample-in-chunk, frame),
    cast fp8.  mvh[c][p, n1, kt, F] = wave[c*64 + F//98, 480*(F%98)
    + (2p+kt)*128 + n1]."""
    dftm, fbd = _host_constants()
    x8 = waveform.astype(ml_dtypes.float8_e4m3)
    fr = np.ascontiguousarray(np.lib.stride_tricks.as_strided(
        x8, (B, NFRAMES, NCHUNK * 128), (SR, HOP, 1)))
    mvh = np.ascontiguousarray(
        fr.reshape(NCORES, FTOT, NPAIR, 2, 128).transpose(0, 2, 4, 3, 1))
    return [
        {"mvh": mvh[c], "dftm": dftm, "fbd": fbd}
        for c in range(NCORES)
    ]


def _assemble(results):
    """Device emits [mel=128, frames=FTOT] per core; transpose to
    (wv, frame, mel) and add the constant pad rows 98..127."""
    out = np.full((B, TFRAMES, NMEL), PADV, dtype=np.float32)
    for c in range(NCORES):
        oc = np.asarray(results[c]["out"])           # (128, FTOT)
        out[c * BPC:(c + 1) * BPC, :NFRAMES, :] = \
            oc.reshape(NMEL, BPC, NFRAMES).transpose(1, 2, 0)
    return out


def kernel(waveform):
    waveform = np.ascontiguousarray(np.asarray(waveform, dtype=np.float32))
    assert waveform.shape == (B, SR), waveform.shape
    nc = _build_nc()
    res = bass_utils.run_bass_kernel_spmd(
        nc, _in_maps(waveform), core_ids=list(range(NCORES)), trace=False
    )
    return _assemble(res.results)
